# revision 2
# baseline (speedup 1.0000x reference)
"""Trainium2 Bass kernel for nn_MultiHeadAttention_76510547410991.

Math: out = ((x @ Wq.T + bq) * (v @ Wv.T + bv)) @ Wc.T + bc
(the reference's raw reshapes cancel around the elementwise product).

f32r matmuls (1 col/cycle warm) for all three GEMMs — fp8 DoubleRow was
measured slower (LDWEIGHTS serialization, no FWL).  Improvements over
the 147us baseline:

  * C-projection is interleaved into the q/v matmul stream per m-slice
    (lag `clag`), accumulating into 4 persistent PSUM banks (one per
    128-row group; the matmul start-bit zeroes a whole 2KB bank, so
    accumulation groups never share one).  This removes the per-tile
    pipeline bubble where the PE waited on the last pt slices.
  * qb/vb/pt are bf16: the qb*vb multiply hits the DVE 4x path
    (~190ns vs ~660ns per [128,512]); pt/wc bf16 also halves their
    SBUF footprint.  End-to-end error ~6e-3 absmax-rel (gate 2e-2).
  * Bias adds are split ACT/DVE (~22/10 per tile) so neither engine
    co-saturates with the PE (the old baseline had ACT at ~95% of PE).

Per-core PE floor: 8 tiles x 49152 cycles  (~133us at the ~2.96GHz
observed warm rate); engines per tile: PE 16.6us, ACT ~12.5, DVE ~12.5.
"""

import numpy as np

try:
    import concourse.bacc  # noqa: F401
except ImportError:
    import sys

    for p in ("/opt/trn_rl_repo", "/opt/pypackages"):
        if p not in sys.path:
            sys.path.insert(0, p)

import ml_dtypes

H = 8
F = 256
S = 32768
FH = F * H  # 2048
D = F
N_CORES = 8
R = S // N_CORES  # 4096
RT = 512
NM = FH // 128  # 16
NK = F // 128  # 2

_CACHE = {}


def build_program(
    reps=1,
    qp_bufs=2,
    vp_bufs=2,
    pt_bufs=5,
    xv_bufs=3,
    qv_bufs=3,
    o_bufs=4,
    clag=3,  # c-proj trails the q/v stream by this many m-slices
    vb_dve_mod=3,  # vb on DVE except every vb_dve_mod-th -> ACT
    rt=RT,
    probe=None,  # "pe_only" | "noew"
    compile=True,
):
    import concourse.bacc as bacc
    import concourse.mybir as mybir
    import concourse.tile as tile

    f32 = mybir.dt.float32
    f32r = mybir.dt.float32r
    bf16 = mybir.dt.bfloat16
    Act_Id = mybir.ActivationFunctionType.Identity

    nc = bacc.Bacc(
        "TRN2",
        target_bir_lowering=False,
        debug=False,
        enable_asserts=False,
        num_devices=N_CORES,
    )

    x_d = nc.dram_tensor("xT", [F, R], f32r, kind="ExternalInput").ap()
    v_d = nc.dram_tensor("vT", [F, R], f32r, kind="ExternalInput").ap()
    wq_d = nc.dram_tensor("wqT", [F, FH], f32r, kind="ExternalInput").ap()
    wv_d = nc.dram_tensor("wvT", [F, FH], f32r, kind="ExternalInput").ap()
    wc_d = nc.dram_tensor("wcT", [FH, D], bf16, kind="ExternalInput").ap()
    bq_d = nc.dram_tensor("bq2", [128, NM], f32, kind="ExternalInput").ap()
    bv_d = nc.dram_tensor("bv2", [128, NM], f32, kind="ExternalInput").ap()
    bc_d = nc.dram_tensor("bcb", [128, D], f32, kind="ExternalInput").ap()
    out_d = nc.dram_tensor("out", [R, D], f32, kind="ExternalOutput").ap()

    NT = R // rt

    with tile.TileContext(nc) as tc:
        with (
            tc.tile_pool(name="w", bufs=1) as wpool,
            tc.tile_pool(name="xv", bufs=xv_bufs) as xvpool,
            tc.tile_pool(name="qv", bufs=qv_bufs) as qvpool,
            tc.tile_pool(name="p", bufs=pt_bufs) as ppool,
            tc.tile_pool(name="o", bufs=o_bufs) as opool,
            tc.tile_pool(name="qpsum", bufs=qp_bufs, space="PSUM") as qpsum,
            tc.tile_pool(name="vpsum", bufs=vp_bufs, space="PSUM") as vpsum,
            tc.tile_pool(name="opsum", bufs=1, space="PSUM") as opsum,
        ):

            def load_one(tag, dram, n, k):
                r0 = n * rt
                t = xvpool.tile([128, rt], f32r, tag=f"{tag}{k}")
                nc.sync.dma_start(t[:], dram[k * 128 : (k + 1) * 128, r0 : r0 + rt])
                return t

            # weight pieces: FH split into NQ column chunks so startup DMA
            # interleaves with the first matmuls.
            NQ = 4
            qw = FH // NQ  # 512
            wq_sb = [[None] * NQ for _ in range(NK)]
            wv_sb = [[None] * NQ for _ in range(NK)]

            def load_w(dst, dram, q, k, nm):
                qs = slice(q * qw, (q + 1) * qw)
                t = wpool.tile([128, qw], f32r, tag=f"{nm}{k}q{q}")
                nc.sync.dma_start(t[:], dram[k * 128 : (k + 1) * 128, qs])
                dst[k][q] = t

            wc_sb = [None] * NM

            def load_wc(m):
                t = wpool.tile([128, D], bf16, tag=f"wc{m}")
                nc.sync.dma_start(t[:], wc_d[m * 128 : (m + 1) * 128, :])
                wc_sb[m] = t

            mpq = qw // 128

            def wq_ap(k, m):
                return wq_sb[k][m // mpq][:, (m % mpq) * 128 : (m % mpq + 1) * 128]

            def wv_ap(k, m):
                return wv_sb[k][m // mpq][:, (m % mpq) * 128 : (m % mpq + 1) * 128]

            # ---- startup DMA order: first consumers first ----
            x0 = []
            for k in range(NK):
                x0.append(load_one("x", x_d, 0, k))
                load_w(wq_sb, wq_d, 0, k, "wq")
            bq_sb = wpool.tile([128, NM], f32, tag="bq")
            nc.sync.dma_start(bq_sb[:], bq_d[:, :])
            v0 = []
            for k in range(NK):
                v0.append(load_one("v", v_d, 0, k))
                load_w(wv_sb, wv_d, 0, k, "wv")
            bv_sb = wpool.tile([128, NM], f32, tag="bv")
            nc.sync.dma_start(bv_sb[:], bv_d[:, :])
            load_wc(0)
            load_wc(1)
            for q in range(1, NQ):
                for k in range(NK):
                    load_w(wq_sb, wq_d, q, k, "wq")
                for k in range(NK):
                    load_w(wv_sb, wv_d, q, k, "wv")
                for m in range(2 + (q - 1) * 5, min(2 + q * 5, NM)):
                    load_wc(m)
            for m in range(NM):
                if wc_sb[m] is None:
                    load_wc(m)
            bc_sb = wpool.tile([128, D], f32, tag="bc")
            nc.sync.dma_start(bc_sb[:], bc_d[:, :])
            if probe == "noew":
                pt_dummy = wpool.tile([128, rt], bf16, tag="ptd")
                nc.vector.memset(pt_dummy[:], 1.0)

            for rep in range(reps):
                for n in range(NT):
                    r0 = n * rt
                    if rep == 0 and n == 0:
                        xt, vt = x0, v0
                    else:
                        xt = [load_one("x", x_d, n, k) for k in range(NK)]
                        vt = [load_one("v", v_d, n, k) for k in range(NK)]

                    # One op accumulator per 128-row group, each a full PSUM
                    # bank (start-bit zeroing is bank-granular).
                    ops = [
                        opsum.tile([128, 2 * D], f32, tag=f"op{i}", name=f"op{i}")
                        for i in range(rt // 128)
                    ]
                    pts = [None] * NM

                    def cproj(m):
                        pt = pt_dummy if probe == "noew" else pts[m]
                        for s in range(rt // 128):
                            nc.tensor.matmul(
                                ops[s][:, :D],
                                pt[:, s * 128 : (s + 1) * 128],
                                wc_sb[m][:],
                                start=(m == 0),
                                stop=(m == NM - 1),
                            )

                    for m in range(NM):
                        qp = qpsum.tile([128, rt], f32, tag="qp")
                        for k in range(NK):
                            nc.tensor.matmul(
                                qp[:], wq_ap(k, m), xt[k][:],
                                start=(k == 0), stop=(k == NK - 1),
                            )
                        vp = vpsum.tile([128, rt], f32, tag="vp")
                        for k in range(NK):
                            nc.tensor.matmul(
                                vp[:], wv_ap(k, m), vt[k][:],
                                start=(k == 0), stop=(k == NK - 1),
                            )
                        if probe == "pe_only":
                            continue
                        if probe == "noew":
                            if m >= clag:
                                cproj(m - clag)
                            continue
                        qb = qvpool.tile([128, rt], bf16, tag="qb")
                        nc.scalar.activation(
                            qb[:], qp[:], Act_Id, bias=bq_sb[:, m : m + 1]
                        )
                        vb = qvpool.tile([128, rt], bf16, tag="vb")
                        if m % vb_dve_mod == vb_dve_mod - 1:
                            nc.scalar.activation(
                                vb[:], vp[:], Act_Id, bias=bv_sb[:, m : m + 1]
                            )
                        else:
                            nc.vector.tensor_scalar_add(
                                vb[:], vp[:], bv_sb[:, m : m + 1]
                            )
                        pt = ppool.tile([128, rt], bf16, tag="pt")
                        pts[m] = pt
                        nc.vector.tensor_mul(pt[:], qb[:], vb[:])
                        if m >= clag:
                            cproj(m - clag)
                    if probe == "pe_only":
                        continue
                    for m in range(NM - clag, NM):
                        cproj(m)

                    for s in range(rt // 128):
                        ot = opool.tile([128, D], f32, tag="ot")
                        nc.vector.tensor_add(ot[:], ops[s][:, :D], bc_sb[:])
                        nc.sync.dma_start(
                            out_d[r0 + s * 128 : r0 + (s + 1) * 128, :], ot[:]
                        )

    if compile:
        nc.compile()
    return nc


def prep_in_maps(query_key_input, value, Wq, bq, Wv, bv, Wc, bc):
    x = np.asarray(query_key_input, dtype=np.float32)
    v = np.asarray(value, dtype=np.float32)
    shared = {
        "wqT": np.ascontiguousarray(np.asarray(Wq, np.float32).T),
        "wvT": np.ascontiguousarray(np.asarray(Wv, np.float32).T),
        "wcT": np.ascontiguousarray(
            np.asarray(Wc, np.float32).T.astype(ml_dtypes.bfloat16)
        ),
        "bq2": np.ascontiguousarray(np.asarray(bq, np.float32).reshape(NM, 128).T),
        "bv2": np.ascontiguousarray(np.asarray(bv, np.float32).reshape(NM, 128).T),
        "bcb": np.ascontiguousarray(
            np.broadcast_to(np.asarray(bc, np.float32), (128, D))
        ),
    }
    in_maps = []
    for c in range(N_CORES):
        rows = slice(c * R, (c + 1) * R)
        m = dict(shared)
        m["xT"] = np.ascontiguousarray(x[rows].T)
        m["vT"] = np.ascontiguousarray(v[rows].T)
        in_maps.append(m)
    return in_maps


def run_program(nc, in_maps):
    from concourse import bass_utils

    return bass_utils.run_bass_kernel_spmd(nc, in_maps, core_ids=list(range(N_CORES)))


class _Runner:
    """Cached PJRT executable; repeat kernel() calls skip recompiling."""

    def __init__(self, nc):
        import jax

        import concourse.mybir as mybir
        from concourse.bass2jax import (
            _bass_exec_p,
            install_neuronx_cc_hook,
            partition_id_tensor,
        )
        from jax.sharding import Mesh, NamedSharding, PartitionSpec

        try:
            from jax.experimental.shard_map import shard_map
        except ImportError:
            from jax.shard_map import shard_map

        install_neuronx_cc_hook()
        self.jax = jax
        partition_name = (
            nc.partition_id_tensor.name if nc.partition_id_tensor else None
        )
        in_names = []
        out_names = []
        out_avals = []
        self.out_shapes = {}
        for alloc in nc.m.functions[0].allocations:
            if not isinstance(alloc, mybir.MemoryLocationSet):
                continue
            name = alloc.memorylocations[0].name
            if alloc.kind == "ExternalInput":
                if name != partition_name:
                    in_names.append(name)
            elif alloc.kind == "ExternalOutput":
                shape = tuple(alloc.tensor_shape)
                dtype = mybir.dt.np(alloc.dtype)
                out_names.append(name)
                out_avals.append(jax.core.ShapedArray(shape, dtype))
                self.out_shapes[name] = (shape, dtype)
        self.in_names = in_names
        self.out_names = out_names
        all_in = list(in_names) + list(out_names)
        if partition_name is not None:
            all_in.append(partition_name)

        def _body(*args):
            operands = list(args)
            if partition_name is not None:
                operands.append(partition_id_tensor())
            return tuple(
                _bass_exec_p.bind(
                    *operands,
                    out_avals=tuple(out_avals),
                    in_names=tuple(all_in),
                    out_names=tuple(out_names),
                    lowering_input_output_aliases=(),
                    sim_require_finite=True,
                    sim_require_nnan=True,
                    nc=nc,
                )
            )

        devices = jax.devices()[:N_CORES]
        mesh = Mesh(np.asarray(devices), ("core",))
        n_params = len(in_names)
        specs = (PartitionSpec("core"),) * (n_params + len(out_names))
        self.sharding = NamedSharding(mesh, PartitionSpec("core"))
        self.fn = jax.jit(
            shard_map(
                _body,
                mesh=mesh,
                in_specs=specs,
                out_specs=(PartitionSpec("core"),) * len(out_names),
                check_rep=False,
            ),
            keep_unused=True,
        )

    def __call__(self, in_maps):
        jax = self.jax
        ins = [
            jax.device_put(
                np.concatenate([np.asarray(m[n]) for m in in_maps], axis=0),
                self.sharding,
            )
            for n in self.in_names
        ]
        zouts = [
            jax.device_put(np.zeros((N_CORES * s[0], *s[1:]), d), self.sharding)
            for s, d in (self.out_shapes[n] for n in self.out_names)
        ]
        outs = self.fn(*ins, *zouts)
        res = []
        for c in range(N_CORES):
            dd = {}
            for i, n in enumerate(self.out_names):
                s, _ = self.out_shapes[n]
                dd[n] = np.asarray(outs[i]).reshape(N_CORES, *s)[c]
            res.append(dd)
        return res


def kernel(query_key_input, value, Wq, bq, Wk, bk, Wv, bv, Wc, bc):
    in_maps = prep_in_maps(query_key_input, value, Wq, bq, Wv, bv, Wc, bc)
    if "nc" not in _CACHE:
        _CACHE["nc"] = build_program(reps=1)
    nc = _CACHE["nc"]
    try:
        if "runner" not in _CACHE:
            _CACHE["runner"] = _Runner(nc)
        results = _CACHE["runner"](in_maps)
    except Exception:
        _CACHE.pop("runner", None)
        results = run_program(nc, in_maps).results
    out = np.concatenate([results[c]["out"] for c in range(N_CORES)], axis=0)
    return out
